# revision 15
# baseline (speedup 1.0000x reference)
"""AdderNet BasicBlock (adder conv ×2 + BN + SE + residual) on 8 TRN2 cores.

Data-parallel over batch N=16 -> 2 images per core. Inside each core:
  - adder2d: out[co,p] = -sum_{ci,off} |x[ci, p+off] - w[co,ci,off]|
    * |x - w| tiles [128ci, 2048] computed on ScalarE (Abs(w - x) via
      per-partition bias, fp16 in / fp16 out) and VectorE (fp16
      tensor_scalar subtract at 4x mode + sign-bit clear via bitvec AND),
      co-interleaved so TensorE consumes from both engines.
    * partition reduction + (co,off) accumulation on TensorE via one-hot
      column matmuls into PSUM [128co, 2048pos].
  - BN(+ReLU) folded to per-channel scale/bias, one ACT instr from PSUM.
  - SE gate: DVE reduce -> 2 small matmuls + Relu/Sigmoid.
  - residual: (bn2*gate) + x via scalar_tensor_tensor, then ReLU.

fp16 is used for the |x-w| tiles (inputs quantized to fp16); the
reductions/matmul accumulation/BN/SE all stay fp32. End-to-end error vs
the fp32 reference ~1e-4.
"""

import numpy as np
from itertools import product

import concourse.bacc as bacc
import concourse.bass as bass
import concourse.mybir as mybir
import concourse.tile as tile
from concourse.bass_utils import run_bass_kernel_spmd

F32 = mybir.dt.float32
F16 = mybir.dt.float16
U16 = mybir.dt.uint16
AF = mybir.ActivationFunctionType
ALU = mybir.AluOpType

N_CORES = 8
N, C, H, W = 16, 128, 32, 32
NPC = N // N_CORES          # images per core
HP, WP = H + 2, W + 2       # padded
POS = H * W                 # 1024
FREE = NPC * POS            # 2048 free elems per conv instruction
PADF = NPC * HP * WP        # 2312 flat padded size
KK = 9                      # 3x3
EPS = 1e-5

# co -> engine assignment: True = ScalarE(ACT), False = VectorE(DVE).
N_ACT_COS = 50              # of 128, evenly interleaved
DVE_ABS_MODE = "bitvec"     # "bitvec" | "stt"
ACT_FP8 = True              # ACT-cos emit fp8e4 tiles, paired DoubleRow mms
F8 = mybir.dt.float8e4
PM = mybir.MatmulPerfMode


def _use_act(co: int) -> bool:
    return (co * N_ACT_COS) // 128 != ((co + 1) * N_ACT_COS) // 128


def _src_view(padA, padB, dh, dw):
    off = dh * WP + dw
    if off % 2 == 0:
        return padA[:, :, dh:dh + H, dw:dw + W]
    return padB[:].rearrange(
        "p (i h w) -> p i h w", i=NPC, h=HP, w=WP)[
        :, :, dh:dh + H, dw - 1:dw - 1 + W]


OFFS = list(product(range(3), range(3)))


def _conv_layer(nc, padA, padB, wT, psum, pools, Z16, Z8):
    """One adder conv.

    padA/padB: [128, NPC, HP, WP] fp16, B shifted left by one element so
    odd window offsets stay 4-byte aligned (DVE 4x mode).
    -> psum [128co, FREE] accumulates sum over (ci, off) of |x - w|.
    """
    abs_pool, d_pool, pair_pool, s8_pool, s16_pool = pools
    for co in range(C):
        if _use_act(co) and ACT_FP8:
            # 4 offset-pairs as fp8 DoubleRow + 1 single fp8 matmul set
            for pi in range(4):
                pair = pair_pool.tile([128, 2, NPC, H, W], F8, tag="pair")
                for k in range(2):
                    dh, dw = OFFS[2 * pi + k]
                    o = 2 * pi + k
                    col = wT[:, co * KK + o: co * KK + o + 1]
                    nc.scalar.activation(
                        pair[:, k], _src_view(padA, padB, dh, dw),
                        AF.Abs, bias=col, scale=-1.0)
                pf = pair[:].rearrange("p t i h w -> p t (i h w)")
                lhsT8 = Z8[:, :, 128 - co:256 - co]
                for cc in range(FREE // 512):
                    nc.tensor.matmul(
                        psum[:, 512 * cc:512 * (cc + 1)],
                        lhsT8,
                        pf[:, :, 512 * cc:512 * (cc + 1)],
                        start=(co == 0 and pi == 0),
                        stop=False,
                        perf_mode=PM.DoubleRow,
                    )
            # leftover 9th offset
            dh, dw = OFFS[8]
            col = wT[:, co * KK + 8: co * KK + 9]
            t8 = s8_pool.tile([128, NPC, H, W], F8, tag="s8")
            nc.scalar.activation(t8[:], _src_view(padA, padB, dh, dw),
                                 AF.Abs, bias=col, scale=-1.0)
            t8f = t8[:].rearrange("p i h w -> p (i h w)")
            lhsT8s = Z8[:, 0, 128 - co:256 - co]
            for cc in range(FREE // 512):
                nc.tensor.matmul(
                    psum[:, 512 * cc:512 * (cc + 1)],
                    lhsT8s,
                    t8f[:, 512 * cc:512 * (cc + 1)],
                    start=False, stop=(co == C - 1))
            continue
        if _use_act(co):
            # non-fp8 ACT path (ACT_FP8 False)
            lhsT = Z16[:, 128 - co:256 - co]
            for o, (dh, dw) in enumerate(OFFS):
                col = wT[:, co * KK + o: co * KK + o + 1]
                t = abs_pool.tile([128, NPC, H, W], F16, tag="abs")
                nc.scalar.activation(t[:], _src_view(padA, padB, dh, dw),
                                     AF.Abs, bias=col, scale=-1.0)
                tf = t[:].rearrange("p i h w -> p (i h w)")
                for cc in range(FREE // 512):
                    nc.tensor.matmul(
                        psum[:, 512 * cc:512 * (cc + 1)], lhsT,
                        tf[:, 512 * cc:512 * (cc + 1)],
                        start=(co == 0 and o == 0), stop=False)
            continue
        # DVE path: subtract quads of offsets, one in-place sign-clear AND
        lhsT = Z16[:, 128 - co:256 - co]
        for qi in range(3):
            ks = (0, 1, 2, 3) if qi < 2 else (0,)
            if qi < 2:
                d4 = d_pool.tile([128, 4, NPC * H * W], F16, tag="d")
            else:
                d4 = s16_pool.tile([128, 1, NPC * H * W], F16, tag="s16")
            for k in ks:
                o = 4 * qi + k
                dh, dw = OFFS[o]
                col = wT[:, co * KK + o: co * KK + o + 1]
                nc.vector.tensor_scalar(
                    d4[:, k], _src_view(padA, padB, dh, dw), col, None,
                    op0=ALU.subtract, op1=ALU.bypass)
            nwords = len(ks) * NPC * H * W
            dflat = d4[:].rearrange("p t f -> p (t f)")[:, :nwords]
            nc.vector.tensor_scalar(
                dflat.bitcast(U16), dflat.bitcast(U16), 0x7FFF, None,
                op0=ALU.bitwise_and, op1=ALU.bypass)
            for k in ks:
                o = 4 * qi + k
                for cc in range(FREE // 512):
                    nc.tensor.matmul(
                        psum[:, 512 * cc:512 * (cc + 1)], lhsT,
                        d4[:, k, 512 * cc:512 * (cc + 1)],
                        start=(co == 0 and o == 0),
                        stop=(co == C - 1 and o == KK - 1),
                    )


def _mk_shifted_pair(nc, pool, tag, src_flat_f32=None):
    """Allocate fp16 A/B padded tiles."""
    A = pool.tile([128, NPC, HP, WP], F16, tag=tag + "A")
    B = pool.tile([128, PADF], F16, tag=tag + "B")
    return A, B


def _build_nc():
    nc = bacc.Bacc("TRN2", target_bir_lowering=False, debug=False,
                   num_devices=N_CORES)

    x_d = nc.dram_tensor("x", [NPC, C, H, W], F32, kind="ExternalInput")
    wT1_d = nc.dram_tensor("wT1", [C, C * KK], F32, kind="ExternalInput")
    wT2_d = nc.dram_tensor("wT2", [C, C * KK], F32, kind="ExternalInput")
    bnc_d = nc.dram_tensor("bnc", [C, 4], F32, kind="ExternalInput")
    fc1T_d = nc.dram_tensor("fc1T", [C, 8], F32, kind="ExternalInput")
    fc1b_d = nc.dram_tensor("fc1b", [8, 1], F32, kind="ExternalInput")
    fc2T_d = nc.dram_tensor("fc2T", [8, C], F32, kind="ExternalInput")
    fc2b_d = nc.dram_tensor("fc2b", [C, 1], F32, kind="ExternalInput")
    out_d = nc.dram_tensor("out", [NPC, C, H, W], F32, kind="ExternalOutput")

    xa, outa = x_d.ap(), out_d.ap()

    with tile.TileContext(nc) as tc:
        with (
            tc.tile_pool(name="const", bufs=1) as cpool,
            tc.tile_pool(name="pad", bufs=1) as padpool,
            tc.tile_pool(name="absp", bufs=2) as abs_pool,
            tc.tile_pool(name="dp", bufs=4) as d_pool,
            tc.tile_pool(name="s16p", bufs=3) as s16_pool,
            tc.tile_pool(name="pairp", bufs=4) as pair_pool,
            tc.tile_pool(name="s8p", bufs=3) as s8_pool,
            tc.tile_pool(name="misc", bufs=1) as mpool,
            tc.tile_pool(name="psum", bufs=1, space=bass.MemorySpace.PSUM) as pp,
            tc.tile_pool(name="psum_se", bufs=2, space=bass.MemorySpace.PSUM) as pps,
        ):
            # constants
            Z16 = cpool.tile([128, 256], F16, tag="Z16")   # one-hot bank
            nc.vector.memset(Z16[:], 0.0)
            nc.vector.memset(Z16[:, 128:129], 1.0)
            Z8 = cpool.tile([128, 2, 256], F8, tag="Z8")
            nc.vector.memset(Z8[:], 0.0)
            nc.vector.memset(Z8[:, :, 128:129], 1.0)
            pools = (abs_pool, d_pool, pair_pool, s8_pool, s16_pool)
            wT1 = cpool.tile([C, C * KK], F32, tag="wT1")
            nc.sync.dma_start(wT1[:], wT1_d.ap())
            wT2 = cpool.tile([C, C * KK], F32, tag="wT2")
            nc.sync.dma_start(wT2[:], wT2_d.ap())
            bnc = cpool.tile([C, 4], F32, tag="bnc")
            nc.sync.dma_start(bnc[:], bnc_d.ap())
            fc1T = cpool.tile([C, 8], F32, tag="fc1T")
            nc.sync.dma_start(fc1T[:], fc1T_d.ap())
            fc1b = cpool.tile([8, 1], F32, tag="fc1b")
            nc.sync.dma_start(fc1b[:], fc1b_d.ap())
            fc2T = cpool.tile([8, C], F32, tag="fc2T")
            nc.sync.dma_start(fc2T[:], fc2T_d.ap())
            fc2b = cpool.tile([C, 1], F32, tag="fc2b")
            nc.sync.dma_start(fc2b[:], fc2b_d.ap())

            # padded fp32 input (kept for the residual add)
            xpad = padpool.tile([128, NPC, HP, WP], F32, tag="xpad")
            nc.vector.memset(xpad[:], 0.0)
            for i in range(NPC):
                nc.sync.dma_start(xpad[:, i, 1:1 + H, 1:1 + W], xa[i])
            # fp16 A/B copies for the conv reads
            xA = padpool.tile([128, NPC, HP, WP], F16, tag="xA")
            xB = padpool.tile([128, PADF], F16, tag="xB")
            xpf = xpad[:].rearrange("p i h w -> p (i h w)")
            xAf = xA[:].rearrange("p i h w -> p (i h w)")
            nc.vector.tensor_copy(xAf, xpf)
            nc.vector.memset(xB[:, PADF - 1:PADF], 0.0)
            nc.vector.tensor_copy(xB[:, 0:PADF - 1], xpf[:, 1:PADF])

            # ---- conv1 + BN1 + ReLU -> out1 fp16 A/B ----
            with nc.named_scope("conv1"):
                psum1 = pp.tile([128, FREE], F32, tag="big")
                _conv_layer(nc, xA, xB, wT1, psum1, pools, Z16, Z8)
                o1A = padpool.tile([128, NPC, HP, WP], F16, tag="o1A")
                o1B = padpool.tile([128, PADF], F16, tag="o1B")
                nc.vector.memset(o1A[:], 0.0)
                # bn1: relu(-a1 * s + b1); psum holds s = sum|x-w| >= 0
                nc.scalar.activation(
                    o1A[:, :, 1:1 + H, 1:1 + W],
                    psum1[:].rearrange("p (i h w) -> p i h w", i=NPC, h=H, w=W),
                    AF.Relu, bias=bnc[:, 1:2], scale=bnc[:, 0:1])
                o1Af = o1A[:].rearrange("p i h w -> p (i h w)")
                nc.vector.memset(o1B[:, PADF - 1:PADF], 0.0)
                nc.vector.tensor_copy(
                    o1B[:, 0:PADF - 1].bitcast(U16), o1Af[:, 1:PADF].bitcast(U16))

            # ---- conv2 + BN2 -> bn2out fp32 ----
            with nc.named_scope("conv2"):
                psum2 = pp.tile([128, FREE], F32, tag="big")
                _conv_layer(nc, o1A, o1B, wT2, psum2, pools, Z16, Z8)
                bn2out = mpool.tile([128, FREE], F32, tag="bn2out")
                nc.scalar.activation(
                    bn2out[:], psum2[:], AF.Identity,
                    bias=bnc[:, 3:4], scale=bnc[:, 2:3])

            # ---- SE gate ----
            with nc.named_scope("se"):
                pooled = mpool.tile([128, NPC], F32, tag="pooled")
                for i in range(NPC):
                    nc.vector.reduce_sum(
                        pooled[:, i:i + 1], bn2out[:, POS * i:POS * (i + 1)],
                        axis=mybir.AxisListType.X)
                ps_se1 = pps.tile([8, NPC], F32, tag="se")
                # fc1T pre-scaled by 1/POS (mean folded in)
                nc.tensor.matmul(ps_se1[:], fc1T[:], pooled[:],
                                 start=True, stop=True)
                s2 = mpool.tile([8, NPC], F32, tag="s2")
                nc.scalar.activation(s2[:], ps_se1[:], AF.Relu,
                                     bias=fc1b[:, 0:1])
                ps_se2 = pps.tile([128, NPC], F32, tag="se")
                nc.tensor.matmul(ps_se2[:], fc2T[:], s2[:],
                                 start=True, stop=True)
                gate = mpool.tile([128, NPC], F32, tag="gate")
                nc.scalar.activation(gate[:], ps_se2[:], AF.Sigmoid,
                                     bias=fc2b[:, 0:1])

                # ---- residual + final relu + store ----
                outsb = mpool.tile([128, FREE], F32, tag="outsb")
                bn4 = bn2out[:].rearrange("p (i h w) -> p i h w",
                                          i=NPC, h=H, w=W)
                o4 = outsb[:].rearrange("p (i h w) -> p i h w",
                                        i=NPC, h=H, w=W)
                for i in range(NPC):
                    t2 = mpool.tile([128, H, W], F32, tag="t2")
                    nc.vector.scalar_tensor_tensor(
                        t2[:], bn4[:, i], gate[:, i:i + 1],
                        xpad[:, i, 1:1 + H, 1:1 + W],
                        op0=ALU.mult, op1=ALU.add)
                    nc.scalar.activation(o4[:, i], t2[:], AF.Relu)
                    nc.sync.dma_start(outa[i], o4[:, i])

    nc.compile()
    return nc


_NC_CACHE = None


def _get_nc():
    global _NC_CACHE
    if _NC_CACHE is None:
        _NC_CACHE = _build_nc()
    return _NC_CACHE


def _host_prep(inputs):
    f = np.float32
    w1 = np.ascontiguousarray(inputs["w1"], dtype=f)
    w2 = np.ascontiguousarray(inputs["w2"], dtype=f)
    # [co, ci, kh, kw] -> [ci, co*9 + off]
    wT1 = np.ascontiguousarray(w1.transpose(1, 0, 2, 3).reshape(C, C * KK))
    wT2 = np.ascontiguousarray(w2.transpose(1, 0, 2, 3).reshape(C, C * KK))

    def bn_fold(g, b, m, v):
        g, b, m, v = (np.asarray(t, np.float64) for t in (g, b, m, v))
        a = g / np.sqrt(v + EPS)
        return (-a).astype(f), (b - m * a).astype(f)

    s1, b1 = bn_fold(inputs["bn1_gamma"], inputs["bn1_beta"],
                     inputs["bn1_mean"], inputs["bn1_var"])
    s2, b2 = bn_fold(inputs["bn2_gamma"], inputs["bn2_beta"],
                     inputs["bn2_mean"], inputs["bn2_var"])
    bnc = np.ascontiguousarray(np.stack([s1, b1, s2, b2], axis=1))

    fc1T = np.ascontiguousarray(inputs["fc1_w"].astype(f).T / np.float32(POS))
    fc1b = np.ascontiguousarray(inputs["fc1_b"].astype(f).reshape(8, 1))
    fc2T = np.ascontiguousarray(inputs["fc2_w"].astype(f).T)
    fc2b = np.ascontiguousarray(inputs["fc2_b"].astype(f).reshape(C, 1))
    return dict(wT1=wT1, wT2=wT2, bnc=bnc, fc1T=fc1T, fc1b=fc1b,
                fc2T=fc2T, fc2b=fc2b)


def run(inputs, trace=False, tmpdir=None):
    nc = _get_nc()
    shared = _host_prep(inputs)
    x = np.ascontiguousarray(inputs["x"], dtype=np.float32)
    in_maps = []
    for i in range(N_CORES):
        m = dict(shared)
        m["x"] = np.ascontiguousarray(x[i * NPC:(i + 1) * NPC])
        in_maps.append(m)
    res = run_bass_kernel_spmd(nc, in_maps, core_ids=list(range(N_CORES)),
                               trace=trace, tmpdir=tmpdir)
    out = np.concatenate([res.results[i]["out"] for i in range(N_CORES)], 0)
    return out, res


def kernel(**inputs) -> np.ndarray:
    out, _ = run(inputs)
    return out


# revision 17
# speedup vs baseline: 1.1931x; 1.1931x over previous
"""AdderNet BasicBlock (adder conv ×2 + BN + SE + residual) on 8 TRN2 cores.

Data-parallel over batch N=16 -> 2 images per core. Inside each core:
  - adder2d: out[co,p] = -sum_{ci,off} |x[ci, p+off] - w[co,ci,off]|
    * |x - w| tiles [128ci, 2048] computed on ScalarE (Abs(w - x) via
      per-partition bias, fp16 in / fp16 out) and VectorE (fp16
      tensor_scalar subtract at 4x mode + sign-bit clear via bitvec AND),
      co-interleaved so TensorE consumes from both engines.
    * partition reduction + (co,off) accumulation on TensorE via one-hot
      column matmuls into PSUM [128co, 2048pos].
  - BN(+ReLU) folded to per-channel scale/bias, one ACT instr from PSUM.
  - SE gate: DVE reduce -> 2 small matmuls + Relu/Sigmoid.
  - residual: (bn2*gate) + x via scalar_tensor_tensor, then ReLU.

fp16 is used for the |x-w| tiles (inputs quantized to fp16); the
reductions/matmul accumulation/BN/SE all stay fp32. End-to-end error vs
the fp32 reference ~1e-4.
"""

import numpy as np
from itertools import product

import concourse.bacc as bacc
import concourse.bass as bass
import concourse.mybir as mybir
import concourse.tile as tile
from concourse.bass_utils import run_bass_kernel_spmd

F32 = mybir.dt.float32
F16 = mybir.dt.float16
U16 = mybir.dt.uint16
AF = mybir.ActivationFunctionType
ALU = mybir.AluOpType

N_CORES = 8
N, C, H, W = 16, 128, 32, 32
NPC = N // N_CORES          # images per core
HP, WP = H + 2, W + 2       # padded
POS = H * W                 # 1024
FREE = NPC * POS            # 2048 free elems per conv instruction
PADF = NPC * HP * WP        # 2312 flat padded size
KK = 9                      # 3x3
EPS = 1e-5

# co -> engine assignment: True = ScalarE(ACT), False = VectorE(DVE).
N_ACT_COS = 50              # of 128, evenly interleaved
DVE_ABS_MODE = "bitvec"     # "bitvec" | "stt"
ACT_FP8 = True              # ACT-cos emit fp8e4 tiles, paired DoubleRow mms
F8 = mybir.dt.float8e4
PM = mybir.MatmulPerfMode


def _use_act(co: int) -> bool:
    return (co * N_ACT_COS) // 128 != ((co + 1) * N_ACT_COS) // 128


def _src_view(padA, padB, dh, dw):
    off = dh * WP + dw
    if off % 2 == 0:
        return padA[:, :, dh:dh + H, dw:dw + W]
    return padB[:].rearrange(
        "p (i h w) -> p i h w", i=NPC, h=HP, w=WP)[
        :, :, dh:dh + H, dw - 1:dw - 1 + W]


OFFS = list(product(range(3), range(3)))


def _conv_layer(nc, padA, padB, wT, psum, pools, Z16, Z8):
    """One adder conv.

    padA/padB: [128, NPC, HP, WP] fp16, B shifted left by one element so
    odd window offsets stay 4-byte aligned (DVE 4x mode).
    -> psum [128co, FREE] accumulates sum over (ci, off) of |x - w|.
    """
    abs_pool, d_pool, pair_pool, s8_pool, s16_pool = pools
    for co in range(C):
        if _use_act(co) and ACT_FP8:
            # 4 offset-pairs as fp8 DoubleRow + 1 single fp8 matmul set
            for pi in range(4):
                pair = pair_pool.tile([128, 2, NPC, H, W], F8, tag="pair")
                for k in range(2):
                    dh, dw = OFFS[2 * pi + k]
                    o = 2 * pi + k
                    col = wT[:, co * KK + o: co * KK + o + 1]
                    nc.scalar.activation(
                        pair[:, k], _src_view(padA, padB, dh, dw),
                        AF.Abs, bias=col, scale=-1.0)
                pf = pair[:].rearrange("p t i h w -> p t (i h w)")
                lhsT8 = Z8[:, :, 128 - co:256 - co]
                for cc in range(FREE // 512):
                    nc.tensor.matmul(
                        psum[:, 512 * cc:512 * (cc + 1)],
                        lhsT8,
                        pf[:, :, 512 * cc:512 * (cc + 1)],
                        start=(co == 0 and pi == 0),
                        stop=False,
                        perf_mode=PM.DoubleRow,
                    )
            # leftover 9th offset
            dh, dw = OFFS[8]
            col = wT[:, co * KK + 8: co * KK + 9]
            t8 = s8_pool.tile([128, NPC, H, W], F8, tag="s8")
            nc.scalar.activation(t8[:], _src_view(padA, padB, dh, dw),
                                 AF.Abs, bias=col, scale=-1.0)
            t8f = t8[:].rearrange("p i h w -> p (i h w)")
            lhsT8s = Z8[:, 0, 128 - co:256 - co]
            for cc in range(FREE // 512):
                nc.tensor.matmul(
                    psum[:, 512 * cc:512 * (cc + 1)],
                    lhsT8s,
                    t8f[:, 512 * cc:512 * (cc + 1)],
                    start=False, stop=(co == C - 1))
            continue
        if _use_act(co):
            # non-fp8 ACT path (ACT_FP8 False)
            lhsT = Z16[:, 128 - co:256 - co]
            for o, (dh, dw) in enumerate(OFFS):
                col = wT[:, co * KK + o: co * KK + o + 1]
                t = abs_pool.tile([128, NPC, H, W], F16, tag="abs")
                nc.scalar.activation(t[:], _src_view(padA, padB, dh, dw),
                                     AF.Abs, bias=col, scale=-1.0)
                tf = t[:].rearrange("p i h w -> p (i h w)")
                for cc in range(FREE // 512):
                    nc.tensor.matmul(
                        psum[:, 512 * cc:512 * (cc + 1)], lhsT,
                        tf[:, 512 * cc:512 * (cc + 1)],
                        start=(co == 0 and o == 0), stop=False)
            continue
        # DVE path: subtract pairs of offsets, one sign-clear AND per pair
        lhsT = Z16[:, 128 - co:256 - co]
        for pi in range(5):
            ks = (0, 1) if pi < 4 else (0,)
            d2 = d_pool.tile([128, 2, NPC * H * W], F16, tag="d")
            for k in ks:
                o = 2 * pi + k
                dh, dw = OFFS[o]
                col = wT[:, co * KK + o: co * KK + o + 1]
                nc.vector.tensor_scalar(
                    d2[:, k], _src_view(padA, padB, dh, dw), col, None,
                    op0=ALU.subtract, op1=ALU.bypass)
            t2 = abs_pool.tile([128, 2, NPC * H * W], F16, tag="abs")
            nwords = len(ks) * NPC * H * W
            nc.vector.tensor_scalar(
                t2[:].rearrange("p t f -> p (t f)")[:, :nwords].bitcast(U16),
                d2[:].rearrange("p t f -> p (t f)")[:, :nwords].bitcast(U16),
                0x7FFF, None, op0=ALU.bitwise_and, op1=ALU.bypass)
            for k in ks:
                o = 2 * pi + k
                for cc in range(FREE // 512):
                    nc.tensor.matmul(
                        psum[:, 512 * cc:512 * (cc + 1)], lhsT,
                        t2[:, k, 512 * cc:512 * (cc + 1)],
                        start=(co == 0 and o == 0),
                        stop=(co == C - 1 and o == KK - 1),
                    )


def _build_nc():
    nc = bacc.Bacc("TRN2", target_bir_lowering=False, debug=False,
                   num_devices=N_CORES)

    x_d = nc.dram_tensor("x", [NPC, C, H, W], F32, kind="ExternalInput")
    wT1_d = nc.dram_tensor("wT1", [C, C * KK], F32, kind="ExternalInput")
    wT2_d = nc.dram_tensor("wT2", [C, C * KK], F32, kind="ExternalInput")
    bnc_d = nc.dram_tensor("bnc", [C, 4], F32, kind="ExternalInput")
    fc1T_d = nc.dram_tensor("fc1T", [C, 8], F32, kind="ExternalInput")
    fc1b_d = nc.dram_tensor("fc1b", [8, 1], F32, kind="ExternalInput")
    fc2T_d = nc.dram_tensor("fc2T", [8, C], F32, kind="ExternalInput")
    fc2b_d = nc.dram_tensor("fc2b", [C, 1], F32, kind="ExternalInput")
    out_d = nc.dram_tensor("out", [NPC, C, H, W], F32, kind="ExternalOutput")

    xa, outa = x_d.ap(), out_d.ap()

    with tile.TileContext(nc) as tc:
        with (
            tc.tile_pool(name="const", bufs=1) as cpool,
            tc.tile_pool(name="pad", bufs=1) as padpool,
            tc.tile_pool(name="absp", bufs=5) as abs_pool,
            tc.tile_pool(name="dp", bufs=3) as d_pool,
            tc.tile_pool(name="s16p", bufs=1) as s16_pool,
            tc.tile_pool(name="pairp", bufs=4) as pair_pool,
            tc.tile_pool(name="s8p", bufs=3) as s8_pool,
            tc.tile_pool(name="misc", bufs=1) as mpool,
            tc.tile_pool(name="psum", bufs=1, space=bass.MemorySpace.PSUM) as pp,
            tc.tile_pool(name="psum_se", bufs=2, space=bass.MemorySpace.PSUM) as pps,
        ):
            # constants
            Z16 = cpool.tile([128, 256], F16, tag="Z16")   # one-hot bank
            nc.vector.memset(Z16[:], 0.0)
            nc.vector.memset(Z16[:, 128:129], 1.0)
            Z8 = cpool.tile([128, 2, 256], F8, tag="Z8")
            nc.vector.memset(Z8[:], 0.0)
            nc.vector.memset(Z8[:, :, 128:129], 1.0)
            pools = (abs_pool, d_pool, pair_pool, s8_pool, s16_pool)
            wT1 = cpool.tile([C, C * KK], F32, tag="wT1")
            nc.sync.dma_start(wT1[:], wT1_d.ap())
            wT2 = cpool.tile([C, C * KK], F32, tag="wT2")
            nc.sync.dma_start(wT2[:], wT2_d.ap())
            bnc = cpool.tile([C, 4], F32, tag="bnc")
            nc.sync.dma_start(bnc[:], bnc_d.ap())
            fc1T = cpool.tile([C, 8], F32, tag="fc1T")
            nc.sync.dma_start(fc1T[:], fc1T_d.ap())
            fc1b = cpool.tile([8, 1], F32, tag="fc1b")
            nc.sync.dma_start(fc1b[:], fc1b_d.ap())
            fc2T = cpool.tile([8, C], F32, tag="fc2T")
            nc.sync.dma_start(fc2T[:], fc2T_d.ap())
            fc2b = cpool.tile([C, 1], F32, tag="fc2b")
            nc.sync.dma_start(fc2b[:], fc2b_d.ap())

            # padded fp32 input (kept for the residual add)
            xpad = padpool.tile([128, NPC, HP, WP], F32, tag="xpad")
            nc.vector.memset(xpad[:], 0.0)
            for i in range(NPC):
                nc.sync.dma_start(xpad[:, i, 1:1 + H, 1:1 + W], xa[i])
            # fp16 A/B copies for the conv reads
            xA = padpool.tile([128, NPC, HP, WP], F16, tag="xA")
            xB = padpool.tile([128, PADF], F16, tag="xB")
            xpf = xpad[:].rearrange("p i h w -> p (i h w)")
            xAf = xA[:].rearrange("p i h w -> p (i h w)")
            nc.vector.tensor_copy(xAf, xpf)
            nc.vector.memset(xB[:, PADF - 1:PADF], 0.0)
            nc.vector.tensor_copy(xB[:, 0:PADF - 1], xpf[:, 1:PADF])

            # ---- conv1 + BN1 + ReLU -> out1 fp16 A/B ----
            with nc.named_scope("conv1"):
                psum1 = pp.tile([128, FREE], F32, tag="big")
                _conv_layer(nc, xA, xB, wT1, psum1, pools, Z16, Z8)
                o1A = padpool.tile([128, NPC, HP, WP], F16, tag="o1A")
                o1B = padpool.tile([128, PADF], F16, tag="o1B")
                nc.vector.memset(o1A[:], 0.0)
                # bn1: relu(-a1 * s + b1); psum holds s = sum|x-w| >= 0
                nc.scalar.activation(
                    o1A[:, :, 1:1 + H, 1:1 + W],
                    psum1[:].rearrange("p (i h w) -> p i h w", i=NPC, h=H, w=W),
                    AF.Relu, bias=bnc[:, 1:2], scale=bnc[:, 0:1])
                o1Af = o1A[:].rearrange("p i h w -> p (i h w)")
                nc.vector.memset(o1B[:, PADF - 1:PADF], 0.0)
                nc.vector.tensor_copy(
                    o1B[:, 0:PADF - 1].bitcast(U16), o1Af[:, 1:PADF].bitcast(U16))

            # ---- conv2 + BN2 -> bn2out fp32 ----
            with nc.named_scope("conv2"):
                psum2 = pp.tile([128, FREE], F32, tag="big")
                _conv_layer(nc, o1A, o1B, wT2, psum2, pools, Z16, Z8)
                bn2out = mpool.tile([128, FREE], F32, tag="bn2out")
                nc.scalar.activation(
                    bn2out[:], psum2[:], AF.Identity,
                    bias=bnc[:, 3:4], scale=bnc[:, 2:3])

            # ---- SE gate ----
            with nc.named_scope("se"):
                pooled = mpool.tile([128, NPC], F32, tag="pooled")
                for i in range(NPC):
                    nc.vector.reduce_sum(
                        pooled[:, i:i + 1], bn2out[:, POS * i:POS * (i + 1)],
                        axis=mybir.AxisListType.X)
                ps_se1 = pps.tile([8, NPC], F32, tag="se")
                # fc1T pre-scaled by 1/POS (mean folded in)
                nc.tensor.matmul(ps_se1[:], fc1T[:], pooled[:],
                                 start=True, stop=True)
                s2 = mpool.tile([8, NPC], F32, tag="s2")
                nc.scalar.activation(s2[:], ps_se1[:], AF.Relu,
                                     bias=fc1b[:, 0:1])
                ps_se2 = pps.tile([128, NPC], F32, tag="se")
                nc.tensor.matmul(ps_se2[:], fc2T[:], s2[:],
                                 start=True, stop=True)
                gate = mpool.tile([128, NPC], F32, tag="gate")
                nc.scalar.activation(gate[:], ps_se2[:], AF.Sigmoid,
                                     bias=fc2b[:, 0:1])

                # ---- residual + final relu + store ----
                outsb = mpool.tile([128, FREE], F32, tag="outsb")
                bn4 = bn2out[:].rearrange("p (i h w) -> p i h w",
                                          i=NPC, h=H, w=W)
                o4 = outsb[:].rearrange("p (i h w) -> p i h w",
                                        i=NPC, h=H, w=W)
                for i in range(NPC):
                    t2 = mpool.tile([128, H, W], F32, tag="t2")
                    nc.vector.scalar_tensor_tensor(
                        t2[:], bn4[:, i], gate[:, i:i + 1],
                        xpad[:, i, 1:1 + H, 1:1 + W],
                        op0=ALU.mult, op1=ALU.add)
                    nc.scalar.activation(o4[:, i], t2[:], AF.Relu)
                    nc.sync.dma_start(outa[i], o4[:, i])

    nc.compile()
    return nc


_NC_CACHE = None


def _get_nc():
    global _NC_CACHE
    if _NC_CACHE is None:
        _NC_CACHE = _build_nc()
    return _NC_CACHE


def _host_prep(inputs):
    f = np.float32
    w1 = np.ascontiguousarray(inputs["w1"], dtype=f)
    w2 = np.ascontiguousarray(inputs["w2"], dtype=f)
    # [co, ci, kh, kw] -> [ci, co*9 + off]
    wT1 = np.ascontiguousarray(w1.transpose(1, 0, 2, 3).reshape(C, C * KK))
    wT2 = np.ascontiguousarray(w2.transpose(1, 0, 2, 3).reshape(C, C * KK))

    def bn_fold(g, b, m, v):
        g, b, m, v = (np.asarray(t, np.float64) for t in (g, b, m, v))
        a = g / np.sqrt(v + EPS)
        return (-a).astype(f), (b - m * a).astype(f)

    s1, b1 = bn_fold(inputs["bn1_gamma"], inputs["bn1_beta"],
                     inputs["bn1_mean"], inputs["bn1_var"])
    s2, b2 = bn_fold(inputs["bn2_gamma"], inputs["bn2_beta"],
                     inputs["bn2_mean"], inputs["bn2_var"])
    bnc = np.ascontiguousarray(np.stack([s1, b1, s2, b2], axis=1))

    fc1T = np.ascontiguousarray(inputs["fc1_w"].astype(f).T / np.float32(POS))
    fc1b = np.ascontiguousarray(inputs["fc1_b"].astype(f).reshape(8, 1))
    fc2T = np.ascontiguousarray(inputs["fc2_w"].astype(f).T)
    fc2b = np.ascontiguousarray(inputs["fc2_b"].astype(f).reshape(C, 1))
    return dict(wT1=wT1, wT2=wT2, bnc=bnc, fc1T=fc1T, fc1b=fc1b,
                fc2T=fc2T, fc2b=fc2b)


def run(inputs, trace=False, tmpdir=None):
    nc = _get_nc()
    shared = _host_prep(inputs)
    x = np.ascontiguousarray(inputs["x"], dtype=np.float32)
    in_maps = []
    for i in range(N_CORES):
        m = dict(shared)
        m["x"] = np.ascontiguousarray(x[i * NPC:(i + 1) * NPC])
        in_maps.append(m)
    res = run_bass_kernel_spmd(nc, in_maps, core_ids=list(range(N_CORES)),
                               trace=trace, tmpdir=tmpdir)
    out = np.concatenate([res.results[i]["out"] for i in range(N_CORES)], 0)
    return out, res


def kernel(**inputs) -> np.ndarray:
    out, _ = run(inputs)
    return out
